# revision 22
# baseline (speedup 1.0000x reference)
"""Trainium2 Bass kernel for nn_CrossLayer (DCN-style cross stack).

Reference semantics (B=16384, D=1024, L=8):
    out_0 = x
    s_i = einsum('bd,d->b', out_i, W[i])
    out_{i+1} = x * s_i[:, None] + b[i] + x

Algebraic collapse: out = x * rho_8 + b[L-1] with
    rho_1 = u_0 + 1,   rho_{l+1} = rho_l * u_l + c_l
    u_l[r] = <x[r, :], W[l]>          (U = x @ W.T, [B, L])
    c_l = <b[l-1], W[l]> + 1          (weights-only scalars)

Device pipeline (per 256-row tile, rows pair-packed 2/partition):
    1. DMA x tile [128, 2048] f32 (8KB contiguous per partition)
    2. ACT pack -> fp16 [128, (h, d)] CONTIGUOUS; u32 word = d-adjacent pair
    3. PE transposes u32 chunks (8x 128x128 per tile) -> per-(h,c) slots
    4. PE matmul per (c, s-phase): [Uh; Ul*2^11]^T += w[c,s].T @ xT cols,
       c-major over tile pairs so each weight load serves 2 tiles.
       fp16 hi/lo split of W kills W's rounding error; x error fp16 2^-11.
    5. uf = M-matmul: [128, 8] u per row-half via ut.T @ M, M = [I; 2^-11 I]
       (constant from host) -- replaces small transposes + combines
    6. rho recurrence vectorized across 8 row-halves (4 tiles) as [128, 8]
       elementwise DVE ops (tensor_tensor_scan is latency-serial ~166ns/el)
    7. y = x * rho + b7 emitted bf16 (r=0 one DVE op; r=1 ACT mul + DVE
       add); gpsimd SWDGE store (host upcasts; ~2^-9 err << 2e-2 gate)

bf16 y stores halve write traffic: 8MB in + 4MB out per core ~= 31us DMA
floor at ~400GB/s. u32 d-pair packing halves PE transpose count vs fp32.

Sharding: data-parallel over batch; 8 cores x 2048 rows.
"""

import numpy as np

import concourse.bacc as bacc
import concourse.tile as tile
from concourse import mybir
from concourse.bass_utils import run_bass_kernel_spmd
from concourse.masks import make_identity

N_CORES = 8
B, D, L = 16384, 1024, 8
RPC = B // N_CORES          # rows per core (2048)
TPR = 2                     # rows packed per partition
NT = RPC // (128 * TPR)     # 256-row tiles per core (8)
NCH = D // 256              # 256-wide d chunks as u32 pairs (4)
N_WARM = 10                 # fp16 warmup matmuls to lift HAM to K=8/8

LAST_RESULTS = None


def _build(cvals):
    """Trace + compile the per-core program. cvals = [c_1..c_{L-1}]."""
    nc = bacc.Bacc("TRN2", target_bir_lowering=False, debug=False)
    f32 = mybir.dt.float32
    bf16 = mybir.dt.bfloat16
    f16 = mybir.dt.float16
    u32 = mybir.dt.uint32

    x_d = nc.dram_tensor("x", [RPC, D], f32, kind="ExternalInput")
    # wt[q, (c4, s2, m16)]: m = [Wh(8) | Wl*2^11(8)] at d = 256c + 2q + s
    wt_d = nc.dram_tensor("wt", [128, 128], f16, kind="ExternalInput")
    m_d = nc.dram_tensor("mcmb", [2 * L, L], f32, kind="ExternalInput")
    b7_d = nc.dram_tensor("b7r", [128, D], bf16, kind="ExternalInput")
    y_d = nc.dram_tensor("y", [RPC, D], bf16, kind="ExternalOutput")

    # row-pair tile views: [t][p, (r d)] -- 8KB contiguous per partition
    x_tile = x_d.ap().rearrange("(t p r) d -> t p (r d)", p=128, r=TPR)
    y_tile = y_d.ap().rearrange("(t p r) d -> t p (r d)", p=128, r=TPR)

    with tile.TileContext(nc) as tc:
        with (
            tc.tile_pool(name="const", bufs=1) as cpool,
            tc.tile_pool(name="xf", bufs=5) as xfpool,
            tc.tile_pool(name="pk", bufs=3) as pkpool,
            tc.tile_pool(name="xt", bufs=3) as xtpool,
            tc.tile_pool(name="yt", bufs=3) as ytpool,
            tc.tile_pool(name="small", bufs=8) as spool,
            tc.tile_pool(name="pst", bufs=4, space="PSUM") as pst,
            tc.tile_pool(name="psu", bufs=2, space="PSUM") as psu,
            tc.tile_pool(name="puf", bufs=2, space="PSUM") as puf,
        ):
            # --- first x data on the wire before anything else ---
            xf_pre = []
            for t in range(3):
                xf = xfpool.tile([128, TPR * D], f32, tag="xf")
                nc.sync.dma_start(out=xf[:], in_=x_tile[t])
                xf_pre.append(xf)

            # --- warmup: dense fp16 matmuls during initial DMA window ---
            dummy = cpool.tile([128, 256], f16)
            nc.gpsimd.memset(dummy[:], 0.0)
            for i in range(N_WARM):
                pw = pst.tile([128, 2, 128], f32, tag="pst")
                nc.tensor.matmul(
                    pw[:].rearrange("p a b -> p (a b)"), dummy[:, 0:128], dummy[:],
                    start=True, stop=True,
                )

            # --- constants ---
            ident = cpool.tile([128, 128], f32)
            make_identity(nc, ident[:])
            wt_sb = cpool.tile([128, NCH, 2, 2 * L], f16)
            nc.sync.dma_start(
                out=wt_sb[:],
                in_=wt_d.ap().rearrange("q (c s m) -> q c s m", c=NCH, s=2),
            )
            mc_sb = cpool.tile([2 * L, L], f32)
            nc.sync.dma_start(out=mc_sb[:], in_=m_d[:, :])
            b7_sb = cpool.tile([128, D], bf16)
            nc.sync.dma_start(out=b7_sb[:], in_=b7_d[:, :])

            GRT = 4    # tiles per recurrence group (2 MM pairs)
            for g in range(NT // GRT):
                xfs, uts = [], []
                uf = puf.tile([128, GRT * TPR, L], f32, tag="puf")
                for pairi in range(GRT // 2):
                    xts = []
                    for tt in range(2):
                        t = g * GRT + pairi * 2 + tt
                        if t < 3:
                            xf = xf_pre[t]
                        else:
                            xf = xfpool.tile([128, TPR * D], f32, tag="xf")
                            nc.sync.dma_start(out=xf[:], in_=x_tile[t])
                        xfs.append(xf)

                        # pack f32 -> fp16 contiguous [128, (h, d)]
                        pk = pkpool.tile([128, TPR * D], f16, tag="pk")
                        nc.scalar.copy(pk[:], xf[:])

                        # transpose d-pair u32 chunks: slot (h, c)
                        pk32 = pk[:].bitcast(f32)   # [128, 1024] u32-as-f32
                        xT = xtpool.tile([128, TPR * NCH, 128], f32, tag="xt")
                        for half in range(4):
                            psT = pst.tile([128, 2, 128], f32, tag="pst")
                            for cc in range(2):
                                slot = half * 2 + cc
                                nc.tensor.transpose(
                                    psT[:, cc, :],
                                    pk32[:, 128 * slot : 128 * (slot + 1)],
                                    ident[:],
                                )
                            nc.vector.tensor_copy(
                                xT[:, half * 2 : half * 2 + 2, :].bitcast(u32),
                                psT[:].bitcast(u32),
                            )
                        xts.append(xT)

                    # U matmuls, c-major over the pair: one weight load
                    # serves both tiles. psu cols = (h, p) at phase s.
                    ps_ua = psu.tile([2 * L, TPR * 128], f32, tag="psu")
                    ps_ub = psu.tile([2 * L, TPR * 128], f32, tag="psu")
                    ps_us = [ps_ua, ps_ub]
                    for c in range(NCH):
                        for s in range(2):
                            for tt in range(2):
                                # rhs: fp16 view, cols (h: slot-stride, p: 2)
                                rhs = (
                                    xts[tt][:]
                                    .bitcast(f16)
                                    .rearrange(
                                        "q (h c) (p s) -> q c s h p",
                                        h=TPR, s=2,
                                    )[:, c, s, :, :]
                                )
                                nc.tensor.matmul(
                                    ps_us[tt][:],
                                    wt_sb[:, c, s, :],
                                    rhs,
                                    start=(c == 0 and s == 0),
                                    stop=(c == NCH - 1 and s == 1),
                                )
                    for tt in range(2):
                        ti = pairi * 2 + tt
                        ut = spool.tile([2 * L, TPR * 128], f32, tag="ut")
                        nc.scalar.copy(ut[:], ps_us[tt][:])
                        uts.append(ut)
                        # uf[p, half, l] = ut.T @ M  (M = [I8; 2^-11 I8])
                        for h in range(TPR):
                            nc.tensor.matmul(
                                uf[:, ti * TPR + h, :],
                                ut[:, 128 * h : 128 * (h + 1)],
                                mc_sb[:],
                                start=True, stop=True,
                            )

                # rho recurrence, vectorized over the group's 8 row-halves
                nhalf = GRT * TPR
                rho = spool.tile([128, nhalf], f32, tag="rho")
                nc.vector.tensor_scalar_add(rho[:], uf[:, :, 0], 1.0)
                for l in range(1, L):
                    tmp = spool.tile([128, nhalf], f32, tag="tmp")
                    nc.vector.tensor_tensor(
                        tmp[:], rho[:], uf[:, :, l], mybir.AluOpType.mult
                    )
                    rho = spool.tile([128, nhalf], f32, tag="rho")
                    nc.vector.tensor_scalar_add(rho[:], tmp[:], cvals[l - 1])

                # outputs: y = x * rho + b7 (bf16)
                for tt in range(GRT):
                    t = g * GRT + tt
                    xf = xfs[tt]
                    yt = ytpool.tile([128, TPR * D], bf16, tag="yt")
                    k = tt * TPR
                    nc.vector.scalar_tensor_tensor(
                        yt[:, 0:D], xf[:, 0:D], rho[:, k : k + 1], b7_sb[:],
                        mybir.AluOpType.mult, mybir.AluOpType.add,
                    )
                    th = spool.tile([128, D], bf16, tag="th")
                    nc.scalar.mul(th[:], xf[:, D : 2 * D], rho[:, k + 1 : k + 2])
                    nc.vector.tensor_tensor(
                        yt[:, D : 2 * D], th[:], b7_sb[:], mybir.AluOpType.add
                    )
                    nc.gpsimd.dma_start(out=y_tile[t], in_=yt[:])

    nc.compile()
    return nc


def kernel(x, W, b):
    global LAST_RESULTS
    x = np.ascontiguousarray(np.asarray(x), dtype=np.float32)
    W = np.ascontiguousarray(np.asarray(W), dtype=np.float32)
    b = np.ascontiguousarray(np.asarray(b), dtype=np.float32)
    assert x.shape == (B, D) and W.shape == (L, D) and b.shape == (L, D)

    import ml_dtypes

    cvals = [float(np.dot(b[l - 1].astype(np.float64), W[l].astype(np.float64)) + 1.0)
             for l in range(1, L)]
    # fp16 hi/lo split of W: wt[q, c, s, m] = Wsplit[m][l, 256c + 2q + s]
    wh = W.astype(np.float16)
    wl = ((W.astype(np.float64) - wh.astype(np.float64)) * 2048.0).astype(np.float16)
    wcat = np.concatenate([wh, wl], axis=0)             # [16, 1024] fp16
    # index [m, d] -> [q, c, s, m]
    wt = wcat.reshape(2 * L, NCH, 128, 2).transpose(2, 1, 3, 0)
    wt = np.ascontiguousarray(wt.reshape(128, 128))
    mcmb = np.concatenate(
        [np.eye(L, dtype=np.float32), np.eye(L, dtype=np.float32) * (2.0 ** -11)],
        axis=0,
    )                                                    # [16, 8]
    b7r = np.ascontiguousarray(np.broadcast_to(b[L - 1], (128, D))).astype(
        ml_dtypes.bfloat16
    )

    nc = _build(cvals)

    shards = [x[i * RPC : (i + 1) * RPC] for i in range(N_CORES)]
    in_maps = [{"x": s, "wt": wt, "mcmb": mcmb, "b7r": b7r} for s in shards]
    res = run_bass_kernel_spmd(nc, in_maps, core_ids=list(range(N_CORES)))
    LAST_RESULTS = res
    out = np.concatenate([res.results[i]["y"] for i in range(N_CORES)], axis=0)
    return out.astype(np.float32)
